# revision 8
# baseline (speedup 1.0000x reference)
"""Trainium2 Bass kernel for nn_Conv2DLinalgRMSNorm.

Math: out = RMSNormEps(x @ (sum_l conv_w[l])^T / 20) * norm_w
  where RMSNormEps(v) = v / sqrt(sum_h v^2 + eps*H) * sqrt(H)

Strategy (8 NeuronCores, no cross-device collectives).  The host
pre-permutes inputs (bit-identical data movement, part of sharding) so
every DMA is 4 KiB-contiguous per partition and no on-device transpose
is needed anywhere:
  Launch 1 (weight prep, sharded over output-channel columns):
    core i streams its h-major conv slice cwT[l, p, c, o] =
    conv_w[l, i*128+o, c*128+p] (10 MiB) as 20 plain per-layer loads on
    both HWDGE queues.  The 19-layer sum runs as two running-sum chains
    pipelined behind the loads (DVE 13 adds, GpSimd 5 adds on
    early-landing layers); the final DVE merge writes bf16 directly and
    one DMA stores this core's [1024, 128] W_sum^T column slice.
  Launch 2 (token-parallel GEMM + norm):
    core i takes 1024 tokens as xT[tile, p, c, t] = x[t...] tiles, cast
    f32->bf16 during the SWDGE load, then a bf16 GEMM (2 matmuls of
    N=512 per [128h] block, PSUM-accumulated over 8 blocks) with the
    RMSNorm fused on ACT/DVE.  The 1/20 scaling folds into the rsqrt
    bias: out = y * 32 * rsqrt(sum y^2 + 400*eps*H) * norm_w.
"""
import numpy as np

import concourse.bass as bass
import concourse.mybir as mybir
from concourse.tile import TileContext
from concourse import bass_utils

dt = mybir.dt
P = 128
H = 1024
NL = 20
B, S = 2, 4096
TOK = B * S            # 8192
NCORES = 8
TPC = TOK // NCORES    # 1024 tokens per core
EPS = 1e-6
SSQ_BIAS = float(NL * NL * EPS * H)   # 0.4096
NCH = 8                # h blocks of 128
NT = TPC // P          # 8 token tiles per core

_ctr = [0]


def _legalize_waits(nc):
    """This walrus build accepts 1 sync wait per instruction (2 on
    EventSemaphore); split excess waits into standalone waits."""
    def fix_block(blk):
        insts = list(blk.instructions)
        out = []
        changed = False
        for inst in insts:
            si = inst.sync_info
            waits = list(si.on_wait) if si and si.on_wait else []
            cap = 2 if isinstance(inst, mybir.InstEventSemaphore) else 1
            if len(waits) > cap:
                changed = True
                keep = waits[:cap]
                extra = waits[cap:]
                for i in range(0, len(extra), 2):
                    chunk = extra[i:i + 2]
                    _ctr[0] += 1
                    ev = mybir.InstEventSemaphore(
                        name=f"I-waitfix-{_ctr[0]}",
                        engine=inst.engine,
                        ins=[],
                        outs=[],
                        sync_info=mybir.SyncInfo(on_wait=chunk, on_update=[]),
                    )
                    out.append(ev)
                si.on_wait = keep
            out.append(inst)
        if changed:
            blk.instructions = out
        for sub in getattr(blk, "blocks", None) or []:
            fix_block(sub)

    for fn in nc.m.functions:
        for blk in fn.blocks:
            fix_block(blk)


# L1 layer ownership.  GpSimd adds are ~2x slower than DVE, so its
# layers are issued early (its last add retires ~when the DMA stream
# ends); DVE owns the rest, including everything that lands late.
GPS_LAYERS = [1, 3, 5, 7, 9, 11, 13, 15]
DVE_LAYERS = [0, 2, 4, 6, 8, 10, 12, 14, 16, 17, 18, 19]


def build_l1():
    """Weight prep: h-major conv slice [20, 128, 8, 128] -> [1024, 128] bf16."""
    nc = bass.Bass('TRN2', target_bir_lowering=False, debug=False)
    cw = nc.dram_tensor("cw", [NL, P, NCH, P], dt.float32, kind="ExternalInput")
    wtp = nc.dram_tensor("wtp", [H, P], dt.bfloat16, kind="ExternalOutput")
    with TileContext(nc) as tc:
        with (
            tc.tile_pool(name="lay", bufs=1) as lay,
            tc.tile_pool(name="ws", bufs=1) as wsp,
        ):
            bufs = {}
            for j in range(NL):
                b = lay.tile([P, NCH, P], dt.float32, tag=f"b{j}")
                eng = nc.sync if j % 2 == 0 else nc.scalar
                eng.dma_start(b[:], cw[j])
                bufs[j] = b
            gacc = wsp.tile([P, NCH, P], dt.float32, tag="gacc")
            nc.gpsimd.tensor_add(gacc[:], bufs[GPS_LAYERS[0]][:], bufs[GPS_LAYERS[1]][:])
            for l in GPS_LAYERS[2:]:
                nc.gpsimd.tensor_add(gacc[:], gacc[:], bufs[l][:])
            dacc = wsp.tile([P, NCH, P], dt.float32, tag="dacc")
            nc.vector.tensor_add(dacc[:], bufs[DVE_LAYERS[0]][:], bufs[DVE_LAYERS[1]][:])
            for l in DVE_LAYERS[2:]:
                nc.vector.tensor_add(dacc[:], dacc[:], bufs[l][:])
            # final merge rounds to bf16 in one DVE op; one write-out
            wt = wsp.tile([P, NCH, P], dt.bfloat16, tag="wt")
            nc.vector.tensor_add(wt[:], dacc[:], gacc[:])
            nc.sync.dma_start(wtp.rearrange("(c p) o -> p c o", p=P), wt[:])
    _legalize_waits(nc)
    return nc


def build_l2():
    """Token shard GEMM + LinalgRMSNorm: xT pre-permuted on host, wt bf16."""
    nc = bass.Bass('TRN2', target_bir_lowering=False, debug=False)
    xT = nc.dram_tensor("xT", [NT, P, NCH, P], dt.float32, kind="ExternalInput")
    wt = nc.dram_tensor("wt", [H, H], dt.bfloat16, kind="ExternalInput")
    nw = nc.dram_tensor("nw", [H], dt.float32, kind="ExternalInput")
    y = nc.dram_tensor("y", [TPC, H], dt.float32, kind="ExternalOutput")
    with TileContext(nc) as tc:
        with (
            tc.tile_pool(name="w", bufs=1) as wp,
            tc.tile_pool(name="xt", bufs=1) as xtp,
            tc.tile_pool(name="yout", bufs=3) as yp,
            tc.tile_pool(name="sq", bufs=2) as sqp,
            tc.tile_pool(name="stat", bufs=6) as stat,
            tc.tile_pool(name="psum", bufs=3, space="PSUM") as psum,
            tc.tile_pool(name="psumw", bufs=1, space="PSUM") as psumw,
        ):
            # per-chunk weight tiles split across BOTH HWDGE queues and
            # issued first: every tile's matmul group is paced by wt
            # chunk arrival, so wt must not queue behind the x stream
            wt_r = wt.rearrange("(c p) o -> p c o", p=P)
            wtc = []
            for hc in range(NCH):
                w1 = wp.tile([P, H], dt.bfloat16, tag=f"wtc{hc}")
                eng = nc.sync if hc % 2 == 0 else nc.scalar
                eng.dma_start(w1[:], wt_r[:, hc, :])
                wtc.append(w1)
            # all xT cast-loads up front on the SWDGE queue
            xts = []
            for tt in range(NT):
                xb = xtp.tile([P, NCH, P], dt.bfloat16, tag=f"xT{tt}")
                nc.gpsimd.dma_start(xb[:], xT[tt])
                xts.append(xb)
            nwb = wp.tile([P, H], dt.float32, tag="nwb")
            nc.scalar.dma_start(nwb[:], nw[None, :].partition_broadcast(P))
            # prewarm ACT tables (Square, Sqrt) and the PE clock gate
            wz = wp.tile([P, P], dt.bfloat16, tag="wz")
            nc.vector.memset(wz[:], 0.0)
            dumo = stat.tile([P, 1], dt.float32, tag="dumo")
            nc.scalar.activation(dumo[:], wz[:, 0:1],
                                 mybir.ActivationFunctionType.Square)
            nc.scalar.activation(dumo[:], wz[:, 0:1],
                                 mybir.ActivationFunctionType.Sqrt)
            wu = psumw.tile([P, P], dt.float32, tag="wu")
            for _ in range(14):
                nc.tensor.matmul(wu[:], wz[:], wz[:], start=True, stop=True)

            for tt in range(NT):
                xb = xts[tt]
                pt = psum.tile([P, H], dt.float32, tag="pt")
                for hc in range(NCH):
                    st, sp = (hc == 0), (hc == NCH - 1)
                    nc.tensor.matmul(pt[:, bass.ds(0, 512)], xb[:, hc, :],
                                     wtc[hc][:, bass.ds(0, 512)],
                                     start=st, stop=sp)
                    nc.tensor.matmul(pt[:, bass.ds(512, 512)], xb[:, hc, :],
                                     wtc[hc][:, bass.ds(512, 512)],
                                     start=st, stop=sp)

                # ssq over the whole row on ACT (square + free-dim accum)
                sq = sqp.tile([P, H], dt.float32, tag="sq")
                v = stat.tile([P, 1], dt.float32, tag="v")
                nc.scalar.activation(
                    sq[:], pt[:], mybir.ActivationFunctionType.Square,
                    accum_out=v[:],
                )
                vb = stat.tile([P, 1], dt.float32, tag="vb")
                nc.vector.tensor_scalar(
                    vb[:], v[:], SSQ_BIAS, None, mybir.AluOpType.add,
                )
                rv = stat.tile([P, 1], dt.float32, tag="rv")
                nc.vector.reciprocal(rv[:], vb[:])
                s = stat.tile([P, 1], dt.float32, tag="s")
                nc.scalar.activation(
                    s[:], rv[:], mybir.ActivationFunctionType.Sqrt,
                    scale=float(H),
                )

                ysb = yp.tile([P, H], dt.float32, tag="ysb")
                nc.vector.scalar_tensor_tensor(
                    ysb[:], pt[:], s[:], nwb[:],
                    op0=mybir.AluOpType.mult, op1=mybir.AluOpType.mult,
                )
                nc.sync.dma_start(y[bass.ds(tt * P, P), :], ysb[:])
    _legalize_waits(nc)
    return nc


_CACHE = {}


def _get(name, builder):
    if name not in _CACHE:
        _CACHE[name] = builder()
    return _CACHE[name]


def _prep_in1(conv_w):
    """Per-core h-major conv slices: cwT[l, p, c, o] = conv_w[l, oi+o, c*128+p]."""
    in1 = []
    for i in range(NCORES):
        sl = conv_w[:, i * P:(i + 1) * P, :]              # [l, o, h]
        cwt = sl.transpose(0, 2, 1).reshape(NL, NCH, P, P).transpose(0, 2, 1, 3)
        in1.append({"cw": np.ascontiguousarray(cwt)})
    return in1


def _prep_in2(x_flat, wt_full, norm_w):
    """Per-core GEMM inputs; xT[tile, p, c, t] = x[tile*128+t, c*128+p]."""
    in2 = []
    for i in range(NCORES):
        xs = x_flat[i * TPC:(i + 1) * TPC]
        xt = np.ascontiguousarray(
            xs.reshape(NT, P, NCH, P).transpose(0, 3, 2, 1)
        )
        in2.append({"xT": xt, "wt": wt_full, "nw": norm_w})
    return in2


def kernel(hidden_states, conv_w, norm_w):
    in_dtype = hidden_states.dtype
    x_flat = np.asarray(hidden_states, dtype=np.float32).reshape(TOK, H)
    conv_w = np.asarray(conv_w, dtype=np.float32)
    norm_w = np.asarray(norm_w, dtype=np.float32)
    core_ids = list(range(NCORES))

    # Launch 1: weight prep
    nc1 = _get("l1", build_l1)
    res1 = bass_utils.run_bass_kernel_spmd(nc1, _prep_in1(conv_w), core_ids)
    wt_full = np.concatenate([res1.results[i]["wtp"] for i in range(NCORES)], axis=1)

    # Launch 2: GEMM + norm over token shards
    nc2 = _get("l2", build_l2)
    res2 = bass_utils.run_bass_kernel_spmd(
        nc2, _prep_in2(x_flat, wt_full, norm_w), core_ids)
    y = np.concatenate([res2.results[i]["y"] for i in range(NCORES)], axis=0)
    return y.reshape(B, S, H).astype(in_dtype, copy=False)


# revision 9
# speedup vs baseline: 1.0462x; 1.0462x over previous
"""Trainium2 Bass kernel for nn_Conv2DLinalgRMSNorm.

Math: out = RMSNormEps(x @ (sum_l conv_w[l])^T / 20) * norm_w
  where RMSNormEps(v) = v / sqrt(sum_h v^2 + eps*H) * sqrt(H)

Strategy (8 NeuronCores, no cross-device collectives).  The host
pre-permutes inputs (bit-identical data movement, part of sharding) so
every DMA is 4 KiB-contiguous per partition and no on-device transpose
is needed anywhere:
  Launch 1 (weight prep, sharded over output-channel columns):
    core i streams its h-major conv slice cwT[l, p, c, o] =
    conv_w[l, i*128+o, c*128+p] (10 MiB) as 20 plain per-layer loads on
    both HWDGE queues.  The 19-layer sum runs as two running-sum chains
    pipelined behind the loads (DVE 13 adds, GpSimd 5 adds on
    early-landing layers); the final DVE merge writes bf16 directly and
    one DMA stores this core's [1024, 128] W_sum^T column slice.
  Launch 2 (token-parallel GEMM + norm):
    core i takes 1024 tokens as xT[tile, p, c, t] = x[t...] tiles, cast
    f32->bf16 during the SWDGE load, then a bf16 GEMM (2 matmuls of
    N=512 per [128h] block, PSUM-accumulated over 8 blocks) with the
    RMSNorm fused on ACT/DVE.  The 1/20 scaling folds into the rsqrt
    bias: out = y * 32 * rsqrt(sum y^2 + 400*eps*H) * norm_w.
"""
import numpy as np

import concourse.bass as bass
import concourse.mybir as mybir
from concourse.tile import TileContext
from concourse import bass_utils

dt = mybir.dt
P = 128
H = 1024
NL = 20
B, S = 2, 4096
TOK = B * S            # 8192
NCORES = 8
TPC = TOK // NCORES    # 1024 tokens per core
EPS = 1e-6
SSQ_BIAS = float(NL * NL * EPS * H)   # 0.4096
NCH = 8                # h blocks of 128
NT = TPC // P          # 8 token tiles per core

_ctr = [0]


def _legalize_waits(nc):
    """This walrus build accepts 1 sync wait per instruction (2 on
    EventSemaphore); split excess waits into standalone waits."""
    def fix_block(blk):
        insts = list(blk.instructions)
        out = []
        changed = False
        for inst in insts:
            si = inst.sync_info
            waits = list(si.on_wait) if si and si.on_wait else []
            cap = 2 if isinstance(inst, mybir.InstEventSemaphore) else 1
            if len(waits) > cap:
                changed = True
                keep = waits[:cap]
                extra = waits[cap:]
                for i in range(0, len(extra), 2):
                    chunk = extra[i:i + 2]
                    _ctr[0] += 1
                    ev = mybir.InstEventSemaphore(
                        name=f"I-waitfix-{_ctr[0]}",
                        engine=inst.engine,
                        ins=[],
                        outs=[],
                        sync_info=mybir.SyncInfo(on_wait=chunk, on_update=[]),
                    )
                    out.append(ev)
                si.on_wait = keep
            out.append(inst)
        if changed:
            blk.instructions = out
        for sub in getattr(blk, "blocks", None) or []:
            fix_block(sub)

    for fn in nc.m.functions:
        for blk in fn.blocks:
            fix_block(blk)


# L1 layer ownership.  GpSimd adds are ~2x slower than DVE, so its
# layers are issued early (its last add retires ~when the DMA stream
# ends); DVE owns the rest, including everything that lands late.
GPS_LAYERS = [1, 3, 5, 7, 9, 11, 13, 15]
DVE_LAYERS = [0, 2, 4, 6, 8, 10, 12, 14, 16, 17, 18, 19]


def build_l1():
    """Weight prep: h-major conv slice [20, 128, 8, 128] -> [1024, 128] bf16."""
    nc = bass.Bass('TRN2', target_bir_lowering=False, debug=False)
    cw = nc.dram_tensor("cw", [NL, P, NCH, P], dt.float32, kind="ExternalInput")
    wtp = nc.dram_tensor("wtp", [H, P], dt.bfloat16, kind="ExternalOutput")
    with TileContext(nc) as tc:
        with (
            tc.tile_pool(name="lay", bufs=1) as lay,
            tc.tile_pool(name="ws", bufs=1) as wsp,
        ):
            bufs = {}
            for j in range(NL):
                b = lay.tile([P, NCH, P], dt.float32, tag=f"b{j}")
                eng = nc.sync if j % 2 == 0 else nc.scalar
                eng.dma_start(b[:], cw[j])
                bufs[j] = b
            gacc = wsp.tile([P, NCH, P], dt.float32, tag="gacc")
            nc.gpsimd.tensor_add(gacc[:], bufs[GPS_LAYERS[0]][:], bufs[GPS_LAYERS[1]][:])
            for l in GPS_LAYERS[2:]:
                nc.gpsimd.tensor_add(gacc[:], gacc[:], bufs[l][:])
            dacc = wsp.tile([P, NCH, P], dt.float32, tag="dacc")
            nc.vector.tensor_add(dacc[:], bufs[DVE_LAYERS[0]][:], bufs[DVE_LAYERS[1]][:])
            for l in DVE_LAYERS[2:]:
                nc.vector.tensor_add(dacc[:], dacc[:], bufs[l][:])
            # final merge rounds to bf16 in one DVE op; one write-out
            wt = wsp.tile([P, NCH, P], dt.bfloat16, tag="wt")
            nc.vector.tensor_add(wt[:], dacc[:], gacc[:])
            nc.sync.dma_start(wtp.rearrange("(c p) o -> p c o", p=P), wt[:])
    _legalize_waits(nc)
    return nc


def build_l2():
    """Token shard GEMM + LinalgRMSNorm: xT pre-permuted on host, wt bf16."""
    nc = bass.Bass('TRN2', target_bir_lowering=False, debug=False)
    xT = nc.dram_tensor("xT", [NT, P, NCH, P], dt.float32, kind="ExternalInput")
    wt = nc.dram_tensor("wt", [H, H], dt.bfloat16, kind="ExternalInput")
    nw = nc.dram_tensor("nw", [H], dt.float32, kind="ExternalInput")
    y = nc.dram_tensor("y", [TPC, H], dt.float32, kind="ExternalOutput")
    with TileContext(nc) as tc:
        with (
            tc.tile_pool(name="w", bufs=1) as wp,
            tc.tile_pool(name="xt", bufs=1) as xtp,
            tc.tile_pool(name="yout", bufs=3) as yp,
            tc.tile_pool(name="sq", bufs=2) as sqp,
            tc.tile_pool(name="stat", bufs=6) as stat,
            tc.tile_pool(name="psum", bufs=3, space="PSUM") as psum,
            tc.tile_pool(name="psumw", bufs=1, space="PSUM") as psumw,
        ):
            # single SWDGE queue in explicit priority order (SWDGE has
            # one context, so delivery is strictly program-ordered):
            # x0, x1, all wt chunks, then the remaining x tiles.  This
            # paces the PE ramp without SDMA fair-share contention.
            wt_r = wt.rearrange("(c p) o -> p c o", p=P)
            xts, wtc = [], []
            for tt in range(2):
                xb = xtp.tile([P, NCH, P], dt.bfloat16, tag=f"xT{tt}")
                nc.gpsimd.dma_start(xb[:], xT[tt])
                xts.append(xb)
            for hc in range(NCH):
                w1 = wp.tile([P, H], dt.bfloat16, tag=f"wtc{hc}")
                nc.gpsimd.dma_start(w1[:], wt_r[:, hc, :])
                wtc.append(w1)
            for tt in range(2, NT):
                xb = xtp.tile([P, NCH, P], dt.bfloat16, tag=f"xT{tt}")
                nc.gpsimd.dma_start(xb[:], xT[tt])
                xts.append(xb)
            nwb = wp.tile([P, H], dt.float32, tag="nwb")
            nc.scalar.dma_start(nwb[:], nw[None, :].partition_broadcast(P))
            # prewarm ACT tables (Square, Sqrt) and the PE clock gate
            wz = wp.tile([P, P], dt.bfloat16, tag="wz")
            nc.vector.memset(wz[:], 0.0)
            dumo = stat.tile([P, 1], dt.float32, tag="dumo")
            nc.scalar.activation(dumo[:], wz[:, 0:1],
                                 mybir.ActivationFunctionType.Square)
            nc.scalar.activation(dumo[:], wz[:, 0:1],
                                 mybir.ActivationFunctionType.Sqrt)
            wu = psumw.tile([P, P], dt.float32, tag="wu")
            for _ in range(14):
                nc.tensor.matmul(wu[:], wz[:], wz[:], start=True, stop=True)

            for tt in range(NT):
                xb = xts[tt]
                pt = psum.tile([P, H], dt.float32, tag="pt")
                for hc in range(NCH):
                    st, sp = (hc == 0), (hc == NCH - 1)
                    nc.tensor.matmul(pt[:, bass.ds(0, 512)], xb[:, hc, :],
                                     wtc[hc][:, bass.ds(0, 512)],
                                     start=st, stop=sp)
                    nc.tensor.matmul(pt[:, bass.ds(512, 512)], xb[:, hc, :],
                                     wtc[hc][:, bass.ds(512, 512)],
                                     start=st, stop=sp)

                # ssq over the whole row on ACT (square + free-dim accum)
                sq = sqp.tile([P, H], dt.float32, tag="sq")
                v = stat.tile([P, 1], dt.float32, tag="v")
                nc.scalar.activation(
                    sq[:], pt[:], mybir.ActivationFunctionType.Square,
                    accum_out=v[:],
                )
                vb = stat.tile([P, 1], dt.float32, tag="vb")
                nc.vector.tensor_scalar(
                    vb[:], v[:], SSQ_BIAS, None, mybir.AluOpType.add,
                )
                rv = stat.tile([P, 1], dt.float32, tag="rv")
                nc.vector.reciprocal(rv[:], vb[:])
                s = stat.tile([P, 1], dt.float32, tag="s")
                nc.scalar.activation(
                    s[:], rv[:], mybir.ActivationFunctionType.Sqrt,
                    scale=float(H),
                )

                ysb = yp.tile([P, H], dt.float32, tag="ysb")
                nc.vector.scalar_tensor_tensor(
                    ysb[:], pt[:], s[:], nwb[:],
                    op0=mybir.AluOpType.mult, op1=mybir.AluOpType.mult,
                )
                nc.sync.dma_start(y[bass.ds(tt * P, P), :], ysb[:])
    _legalize_waits(nc)
    return nc


_CACHE = {}


def _get(name, builder):
    if name not in _CACHE:
        _CACHE[name] = builder()
    return _CACHE[name]


def _prep_in1(conv_w):
    """Per-core h-major conv slices: cwT[l, p, c, o] = conv_w[l, oi+o, c*128+p]."""
    in1 = []
    for i in range(NCORES):
        sl = conv_w[:, i * P:(i + 1) * P, :]              # [l, o, h]
        cwt = sl.transpose(0, 2, 1).reshape(NL, NCH, P, P).transpose(0, 2, 1, 3)
        in1.append({"cw": np.ascontiguousarray(cwt)})
    return in1


def _prep_in2(x_flat, wt_full, norm_w):
    """Per-core GEMM inputs; xT[tile, p, c, t] = x[tile*128+t, c*128+p]."""
    in2 = []
    for i in range(NCORES):
        xs = x_flat[i * TPC:(i + 1) * TPC]
        xt = np.ascontiguousarray(
            xs.reshape(NT, P, NCH, P).transpose(0, 3, 2, 1)
        )
        in2.append({"xT": xt, "wt": wt_full, "nw": norm_w})
    return in2


def kernel(hidden_states, conv_w, norm_w):
    in_dtype = hidden_states.dtype
    x_flat = np.asarray(hidden_states, dtype=np.float32).reshape(TOK, H)
    conv_w = np.asarray(conv_w, dtype=np.float32)
    norm_w = np.asarray(norm_w, dtype=np.float32)
    core_ids = list(range(NCORES))

    # Launch 1: weight prep
    nc1 = _get("l1", build_l1)
    res1 = bass_utils.run_bass_kernel_spmd(nc1, _prep_in1(conv_w), core_ids)
    wt_full = np.concatenate([res1.results[i]["wtp"] for i in range(NCORES)], axis=1)

    # Launch 2: GEMM + norm over token shards
    nc2 = _get("l2", build_l2)
    res2 = bass_utils.run_bass_kernel_spmd(
        nc2, _prep_in2(x_flat, wt_full, norm_w), core_ids)
    y = np.concatenate([res2.results[i]["y"] for i in range(NCORES)], axis=0)
    return y.reshape(B, S, H).astype(in_dtype, copy=False)
